# revision 23
# baseline (speedup 1.0000x reference)
"""Trainium2 Bass kernel for CapsNet dynamic routing (nn_Capsule_34342558498916).

Full inputs:  u_vecs (64, 64, 1024) f32, W (1024, 32, 64, 16) f32
Full output:  (64, 16, 32) f32  == transpose(v, (0, 2, 1)) of v (B, N, D)

Sharding: capsule dim N=32 split across 8 cores (4 capsules each).  Every core
sees all of u_vecs and its W[:, n_l] slice; the routing loop (softmax over
the full u axis) is then entirely core-local, so no collectives are needed.

Per-core layout:
  u_hat SBUF [128=(par,b), (t, d, n4)] fp16, u = 2t+par  (par packs u-parity
  into the two 64-partition halves; matmuls run in the two disjoint PE
  quadrants via base_partition-derived tile_position).
  Routing contractions run on the PE as accumulating identity/fold matmuls;
  elementwise multiplies on DVE at 2x (fp16/bf16); exp on ACT with fused
  per-partition bias (= -rowmax) reading logits straight from PSUM.
"""

import os
import sys

import numpy as np

for _p in ("/opt/trn_rl_repo", "/opt/pypackages"):
    if _p not in sys.path:
        sys.path.append(_p)

import concourse.bass as bass
from concourse import bacc
import concourse.mybir as mybir
from concourse import tile
from concourse.bass_utils import run_bass_kernel_spmd

# Problem dims (hardcoded per harness contract)
B, C, U, N, D = 64, 64, 1024, 32, 16
NCORES = 8
NL = N // NCORES          # 4 capsules per core
T = U // 2                # 512 u-pairs
DN = D * NL               # 64 = matmul free dim (d, n4)
P = 128
EPS = 1e-8
ROUTINGS = 3

dt = mybir.dt
AF = mybir.ActivationFunctionType
ALU = mybir.AluOpType

_COMPILED = {}
DEBUG = False


def _squash_and_v(nc, sm, s_ps, zi_or_scale, s_sb, v16_dst):
    """s = s_ps * zi (broadcast over d) or * scalar; v = squash(s); write fp16
    v into v16_dst ([64, D, NL] fp16 slice).  Returns v_sb (f32) tile."""
    if isinstance(zi_or_scale, float):
        nc.vector.tensor_scalar_mul(s_sb[:], s_ps[:], zi_or_scale)
    else:
        zi_bc = zi_or_scale[:].unsqueeze(1).broadcast_to([B, D, NL])
        nc.vector.tensor_mul(s_sb[:], s_ps[:], zi_bc)
    ssq = sm.tile([B, D, NL], dt.float32, tag="ssq")
    nc.vector.tensor_mul(ssq[:], s_sb[:], s_sb[:])
    s2 = sm.tile([B, NL], dt.float32, tag="s2")
    # sum over d (innermost after free transpose)
    nc.vector.tensor_reduce(
        s2[:], ssq[:].transpose([0, 2, 1]), axis=mybir.AxisListType.X, op=ALU.add
    )
    s2e = sm.tile([B, NL], dt.float32, tag="s2e")
    nc.vector.tensor_scalar_add(s2e[:], s2[:], EPS)
    rt = sm.tile([B, NL], dt.float32, tag="rt")
    nc.scalar.activation(rt[:], s2e[:], AF.Sqrt)
    den = sm.tile([B, NL], dt.float32, tag="den")
    nc.vector.tensor_scalar_add(den[:], s2e[:], 1.0)
    deni = sm.tile([B, NL], dt.float32, tag="deni")
    nc.vector.reciprocal(deni[:], den[:])
    f = sm.tile([B, NL], dt.float32, tag="f")
    nc.vector.tensor_mul(f[:], rt[:], deni[:])
    v_sb = sm.tile([B, D, NL], dt.float32, tag="v_sb")
    nc.vector.tensor_mul(v_sb[:], s_sb[:], f[:].unsqueeze(1).broadcast_to([B, D, NL]))
    nc.vector.tensor_copy(v16_dst, v_sb[:])
    return v_sb


def _build_program():
    nc = bacc.Bacc()

    ut = nc.dram_tensor("ut", [P, T, B], dt.float32, kind="ExternalInput")
    wt = nc.dram_tensor("wt", [P, T, DN], dt.float32, kind="ExternalInput")
    ident_d = nc.dram_tensor("ident", [P, P], dt.float32, kind="ExternalInput")
    fold_d = nc.dram_tensor("fold", [P, B], dt.float32, kind="ExternalInput")
    out_d = nc.dram_tensor("out", [B, D, NL], dt.float32, kind="ExternalOutput")
    if DEBUG:
        dbg_v1 = nc.dram_tensor("dbg_v1", [B, D, NL], dt.float32, kind="ExternalOutput")
        dbg_v2 = nc.dram_tensor("dbg_v2", [B, D, NL], dt.float32, kind="ExternalOutput")
        dbg_b = nc.dram_tensor("dbg_b", [P, 128, NL], dt.float32, kind="ExternalOutput")
        dbg_e = nc.dram_tensor("dbg_e", [P, T, NL], dt.float32, kind="ExternalOutput")
        dbg_s = nc.dram_tensor("dbg_s", [B, D, NL], dt.float32, kind="ExternalOutput")
        dbg_vb = nc.dram_tensor("dbg_vb", [P, D, NL], dt.float32, kind="ExternalOutput")

    with tile.TileContext(nc) as tc:
        with (
            tc.tile_pool(name="big", bufs=1) as big,
            tc.tile_pool(name="wts", bufs=3) as wts,
            tc.tile_pool(name="prod", bufs=2) as prodp,
            tc.tile_pool(name="sm", bufs=1) as sm,
            tc.tile_pool(name="psB", bufs=2, space="PSUM") as psB,
            tc.tile_pool(name="psS", bufs=1, space="PSUM") as psS,
        ):
            u_hat = big.tile([P, T, D, NL], dt.float32, tag="u_hat")
            e_sb = big.tile([P, T, NL], dt.float16, tag="e_sb")
            ident = sm.tile([P, P], dt.float32, tag="ident")
            fold = sm.tile([P, B], dt.float32, tag="fold")
            vbc = sm.tile([P, D, NL], dt.float32, tag="vbc")
            mneg = sm.tile([P, NL], dt.float32, tag="mneg")
            bmax = sm.tile([P, NL], dt.float32, tag="bmax")
            tmp64 = sm.tile([B, NL], dt.float32, tag="tmp64")
            z_p = sm.tile([P, NL], dt.float32, tag="z_p")
            z_f = sm.tile([B, NL], dt.float32, tag="z_f")
            zi = sm.tile([B, NL], dt.float32, tag="zi")
            s_sb = sm.tile([B, D, NL], dt.float32, tag="s_sb")

            nc.sync.dma_start(ident[:], ident_d[:])
            nc.sync.dma_start(fold[:], fold_d[:])

            # ---------------- Phase 1: u_hat = einsum over c ----------------
            TCH = 16   # t per DMA chunk
            with tc.tile_pool(name="pp", bufs=3, space="PSUM") as pp:
                for ch in range(T // TCH):
                    wt_ch = wts.tile([P, TCH, DN], dt.float32, tag="wt_ch")
                    nc.sync.dma_start(wt_ch[:], wt[:, ch * TCH:(ch + 1) * TCH, :])
                    ut_ch = wts.tile([P, TCH, B], dt.float32, tag="ut_ch")
                    nc.sync.dma_start(ut_ch[:], ut[:, ch * TCH:(ch + 1) * TCH, :])
                    for g in range(TCH // 8):
                        ps = pp.tile([P, 8, DN], dt.float32, tag="pp")
                        for j in range(8):
                            tl = g * 8 + j
                            t = ch * TCH + tl
                            nc.tensor.matmul(
                                ps[0:64, j, :], ut_ch[0:64, tl, :],
                                wt_ch[0:64, tl, :], start=True, stop=True,
                            )
                            nc.tensor.matmul(
                                ps[64:128, j, :], ut_ch[64:128, tl, :],
                                wt_ch[64:128, tl, :], start=True, stop=True,
                            )
                        t0 = ch * TCH + g * 8
                        dst = u_hat[:, t0:t0 + 8, :, :]
                        src = ps[:].rearrange("p e (d n) -> p e d n", d=D)
                        if g % 2 == 0:
                            nc.vector.tensor_copy(dst, src)
                        else:
                            nc.scalar.copy(dst, src)

            # B-logits live in SBUF f32; per-iteration updates accumulate in
            # transient PSUM tiles then fold in via DVE (cross-iteration PSUM
            # accumulation is not reliable).
            b_sb = big.tile([P, T, NL], dt.float32, tag="b_sb")

            # ---------------- Iteration 1: uniform c -> v1 ----------------
            s_ps = psS.tile([B, D, NL], dt.float32, tag="s_ps")
            for t in range(T):
                nc.tensor.matmul(
                    s_ps[:], fold[:], u_hat[:, t, :, :],
                    start=(t == 0), stop=(t == T - 1),
                )
            v1_sb = _squash_and_v(nc, sm, s_ps, 1.0 / U, s_sb, vbc[0:64, :, :])
            nc.sync.dma_start(vbc[64:128, :, :], vbc[0:64, :, :])
            if DEBUG:
                nc.sync.dma_start(dbg_v1[:], v1_sb[:])
                nc.sync.dma_start(dbg_s[:], s_sb[:])

            # ---------------- Iterations 2..3 ----------------
            for it in range(1, ROUTINGS):
                if DEBUG and it == 2:
                    nc.sync.dma_start(dbg_vb[:], vbc[:])
                # b += sum_d u_hat * v   (DVE mult + PE identity-accumulate)
                for q in range(4):
                    for m in range(4):
                        tbase = q * 128 + m * 32
                        pr = prodp.tile([P, 32, D, NL], dt.float32, tag="pr")
                        nc.vector.tensor_mul(
                            pr[:], u_hat[:, tbase:tbase + 32, :, :],
                            vbc[:].unsqueeze(1).broadcast_to([P, 32, D, NL]),
                        )
                        bu = psB.tile([P, 32, NL], dt.float32, tag="bu")
                        for d in range(D):
                            nc.tensor.matmul(
                                bu[:], ident[:], pr[:, :, d, :],
                                start=(d == 0), stop=(d == D - 1),
                            )
                        dst = b_sb[:, tbase:tbase + 32, :]
                        if it == 1:
                            nc.vector.tensor_copy(dst, bu[:])
                        else:
                            nc.vector.tensor_add(dst, dst, bu[:])
                # row max over u (for exp stability)
                nc.vector.tensor_reduce(
                    bmax[:], b_sb[:].transpose([0, 2, 1]),
                    axis=mybir.AxisListType.X, op=ALU.max,
                )
                nc.sync.dma_start(tmp64[:], bmax[64:128, :])
                nc.vector.tensor_tensor(bmax[0:64, :], bmax[0:64, :], tmp64[:], op=ALU.max)
                nc.vector.tensor_scalar_mul(mneg[0:64, :], bmax[0:64, :], -1.0)
                nc.sync.dma_start(mneg[64:128, :], mneg[0:64, :])

                # e = exp(b - rowmax)  (ACT with fused per-partition bias)
                for j in range(NL):
                    nc.scalar.activation(
                        e_sb[:, :, j], b_sb[:, :, j], AF.Exp,
                        bias=mneg[:, j:j + 1], scale=1.0,
                    )
                # Z = sum_u e
                nc.vector.tensor_reduce(
                    z_p[:], e_sb[:].transpose([0, 2, 1]),
                    axis=mybir.AxisListType.X, op=ALU.add,
                )
                nc.sync.dma_start(tmp64[:], z_p[64:128, :])
                nc.vector.tensor_tensor(z_f[:], z_p[0:64, :], tmp64[:], op=ALU.add)
                nc.vector.reciprocal(zi[:], z_f[:])

                # s~ = sum_u e * u_hat  (DVE mult + PE fold-accumulate)
                s_ps = psS.tile([B, D, NL], dt.float32, tag="s_ps")
                for m in range(16):
                    tbase = m * 32
                    pr2 = prodp.tile([P, 32, D, NL], dt.float32, tag="pr")
                    nc.vector.tensor_mul(
                        pr2[:], u_hat[:, tbase:tbase + 32, :, :],
                        e_sb[:, tbase:tbase + 32, :].unsqueeze(2)
                            .broadcast_to([P, 32, D, NL]),
                    )
                    for tl in range(32):
                        nc.tensor.matmul(
                            s_ps[:], fold[:], pr2[:, tl, :, :],
                            start=(m == 0 and tl == 0),
                            stop=(m == 15 and tl == 31),
                        )
                v_sb = _squash_and_v(nc, sm, s_ps, zi, s_sb, vbc[0:64, :, :])
                if it < ROUTINGS - 1:
                    nc.sync.dma_start(vbc[64:128, :, :], vbc[0:64, :, :])
                if DEBUG and it == 2:
                    nc.sync.dma_start(dbg_v2[:], v_sb[:])
                    nc.sync.dma_start(dbg_b[:], b_sb[:, 0:128, :])
                    dbg_e16 = sm.tile([P, T, NL], dt.float32, tag="dbg_e16")
                    nc.vector.tensor_copy(dbg_e16[:], e_sb[:])
                    nc.sync.dma_start(dbg_e[:], dbg_e16[:])

            nc.sync.dma_start(out_d[:], v_sb[:])

    nc.finalize()
    return nc


def _prep_inputs(u_vecs, W):
    """Host-side shard + relayout.  Returns per-core input maps."""
    u32 = np.ascontiguousarray(u_vecs, dtype=np.float32)
    # [(par,c), t, b]:  u = 2t + par
    utc = u32.transpose(1, 2, 0).reshape(C, T, 2, B)           # c, t, par, b
    ut2 = np.ascontiguousarray(utc.transpose(2, 0, 1, 3)).reshape(P, T, B)
    ident = np.eye(P, dtype=np.float32)
    fold = np.tile(np.eye(B, dtype=np.float32), (2, 1))        # [128, 64]
    in_maps = []
    Wf = np.ascontiguousarray(W, dtype=np.float32)
    for k in range(NCORES):
        wk = Wf[:, k * NL:(k + 1) * NL]                        # [U, NL, C, D]
        # [(par,c), t, (d, n4)]
        wkt = wk.transpose(0, 2, 3, 1).reshape(T, 2, C, D * NL)  # t, par, c, dn
        wt2 = np.ascontiguousarray(wkt.transpose(1, 2, 0, 3)).reshape(P, T, DN)
        in_maps.append({"ut": ut2, "wt": wt2, "ident": ident, "fold": fold})
    return in_maps


def kernel(u_vecs: np.ndarray, W: np.ndarray) -> np.ndarray:
    if "nc" not in _COMPILED:
        _COMPILED["nc"] = _build_program()
    nc = _COMPILED["nc"]
    in_maps = _prep_inputs(np.asarray(u_vecs), np.asarray(W))
    res = run_bass_kernel_spmd(nc, in_maps, list(range(NCORES)))
    outs = [np.asarray(res.results[k]["out"]) for k in range(NCORES)]
    return np.concatenate(outs, axis=-1).astype(np.float32)  # (B, D, N)
